# revision 22
# baseline (speedup 1.0000x reference)
"""Trainium2 Bass kernel for nn_DecoderLayer_44487271252363.

Sharding: batch x sequence. 8 cores = 2 batches x 4 chunks of 512 tokens.
Each core computes the full decoder layer for its 512 tokens, streaming the
full (host-folded, bf16-cast) weights from HBM. The only collectives are two
small K/V all-gathers within each batch group of 4 cores.

On-device layout is transposed throughout: activations live as [emb, token]
so every matmul operand is natural (out = lhsT.T @ rhs, contraction on the
partition axis). Softmax runs in [key, query] layout without max-subtraction
(|logits| <= ~5 for this model, verified against the reference), with a
multiplicative 0/1 mask applied after exp. Per-query sums come from
ones-matmuls; the learned sink bias is folded in as exp(sink) added to the
denominator; 1/x and 1/sqrt(x) are computed as exp(-ln(x)) on ScalarE.
"""

import os
from contextlib import ExitStack

import numpy as np
import ml_dtypes

import concourse.bass as bass
import concourse.mybir as mybir
from concourse import bacc
from concourse.tile import TileContext
from concourse import bass_utils

# model dims
B, T, EMB = 2, 2048, 2048
NH, NKV, HD = 16, 4, 128
N_REP = NH // NKV
ROPE_DIM, ROPE_THETA = 64, 10000.0
RH = ROPE_DIM // 2  # 32
WIN = 1024
MLPD = 8192
CS = 2048
SCALING = HD ** -0.5
EPS = 1e-6

# sharding
N_CORES = 8
TPC = 512           # tokens per core
NTT = TPC // 128    # 4 token tiles per core
NET = EMB // 128    # 16 emb tiles
NFT = MLPD // 128   # 64 mlp tiles
NST = CS // 128     # 16 key tiles
GROUPS = [[0, 1, 2, 3], [4, 5, 6, 7]]

F32 = mybir.dt.float32
BF16 = mybir.dt.bfloat16
AF = mybir.ActivationFunctionType

_BUILT = None
LAST_RESULT = None


def _build_nc():
    nc = bacc.Bacc(None, target_bir_lowering=False, num_devices=N_CORES)

    xt_d = nc.declare_dram_parameter("xt", [EMB, TPC], F32, isOutput=False)
    wq_d = nc.declare_dram_parameter("wq", [NH, 128, NET, HD], BF16, isOutput=False)
    wk_d = nc.declare_dram_parameter("wk", [NKV, 128, NET, HD], BF16, isOutput=False)
    wv_d = nc.declare_dram_parameter("wv", [NKV, 128, NET, HD], BF16, isOutput=False)
    wo_d = nc.declare_dram_parameter("wo", [NH, 128, NET, 128], BF16, isOutput=False)
    gate_d = nc.declare_dram_parameter("gate", [NFT, 128, NET, 128], BF16, isOutput=False)
    up_d = nc.declare_dram_parameter("up", [NFT, 128, NET, 128], BF16, isOutput=False)
    down_d = nc.declare_dram_parameter("down", [NET, 128, NFT, 128], BF16, isOutput=False)
    sincos_d = nc.declare_dram_parameter("sincos", [2, ROPE_DIM, TPC], BF16, isOutput=False)
    # multiplicative 0/1 mask, packed [s_in_tile, stile, t]
    mask_d = nc.declare_dram_parameter("maskt", [128, NST, TPC], BF16, isOutput=False)
    sink_d = nc.declare_dram_parameter("sinkexp", [1, NH], F32, isOutput=False)

    out_d = nc.declare_dram_parameter("out", [EMB, TPC], F32, isOutput=True)

    with TileContext(nc) as tc, ExitStack() as top:
        constp = top.enter_context(tc.tile_pool(name="const", bufs=1))
        dramp = top.enter_context(tc.tile_pool(name="dram", bufs=1, space="DRAM"))

        ones_col = constp.tile([128, 1], BF16)
        nc.vector.memset(ones_col, 1.0)
        ones_row = constp.tile([1, 128], F32)
        nc.vector.memset(ones_row, 1.0)
        sink_sb = constp.tile([1, NH], F32)
        nc.sync.dma_start(out=sink_sb, in_=sink_d[:, :])
        eps_sb = constp.tile([1, 1], F32)
        nc.vector.memset(eps_sb, EPS)
        sin_sb = constp.tile([ROPE_DIM, TPC], BF16)
        nc.sync.dma_start(out=sin_sb, in_=sincos_d[0, :, :])
        cos_sb = constp.tile([ROPE_DIM, TPC], BF16)
        nc.sync.dma_start(out=cos_sb, in_=sincos_d[1, :, :])

        # lifetime-scoped pools (stack discipline per SBUF side):
        #   left:  xt(0-5), h1(1-2), at(4-5), h2(6-7) + phase pools
        #   right: q(2-4), wo(3-5), x2(5-8), a(7-8), wdn(7-8)
        xt_es = ExitStack()
        xtp = xt_es.enter_context(tc.tile_pool(name="xt", bufs=1, side="left"))
        h1_es = ExitStack()
        h1p = h1_es.enter_context(tc.tile_pool(name="h1", bufs=1, side="right"))

        xt_all = xtp.tile([128, NET, TPC], F32)
        for g in range(4):
            nc.sync.dma_start(
                out=xt_all[:, g * 4:(g + 1) * 4, :],
                in_=xt_d[g * 512:(g + 1) * 512, :]
                    .rearrange("(et p) t -> p et t", p=128))
        xt_sb = [xt_all[:, e, :] for e in range(NET)]

        h1_sb = [h1p.tile([128, TPC], BF16, name=f"h1_{e}") for e in range(NET)]

        def rmsnorm(src_tiles, dst_tiles, tmpp, psp):
            """dst = src * rsqrt(mean(src^2) + eps), bf16 out."""
            ssq_ps = psp.tile([1, TPC], F32, tag="ssq")
            for e in range(NET):
                sq = tmpp.tile([128, TPC], BF16, tag="sq")
                nc.vector.tensor_mul(sq, src_tiles[e], src_tiles[e])
                nc.tensor.matmul(ssq_ps, ones_col, sq,
                                 start=(e == 0), stop=(e == NET - 1))
            # rs = exp(-0.5 * ln(ssq/EMB + eps))
            lnm = tmpp.tile([1, TPC], F32, tag="lnm")
            nc.scalar.activation(out=lnm, in_=ssq_ps, func=AF.Ln,
                                 scale=1.0 / EMB, bias=eps_sb[:, :])
            rs = tmpp.tile([1, TPC], F32, tag="rs")
            nc.scalar.activation(out=rs, in_=lnm, func=AF.Exp, scale=-0.5)
            rs_ps = psp.tile([128, TPC], F32, tag="rsbc")
            nc.tensor.matmul(rs_ps, ones_row, rs, start=True, stop=True)
            rs_bc = tmpp.tile([128, TPC], F32, tag="rsbcs")
            nc.vector.tensor_copy(out=rs_bc, in_=rs_ps)
            for e in range(NET):
                nc.vector.tensor_mul(dst_tiles[e], src_tiles[e], rs_bc)

        def rope(dst, src_ps, tmpp):
            """dst (bf16 [128,TPC]) = partial rope of src_ps (f32 psum).

            ScalarE stages psum -> bf16 SBUF, then all-bf16 DVE ops run in
            the 2-byte fast mode."""
            stg = tmpp.tile([128, TPC], BF16, tag="ropestg")
            nc.scalar.copy(out=stg, in_=src_ps)
            ta = tmpp.tile([RH, TPC], BF16, tag="ropea")
            tb = tmpp.tile([RH, TPC], BF16, tag="ropeb")
            # out[0:RH] = x0*cos0 - x1*sin1
            nc.vector.tensor_mul(ta, stg[0:RH, :], cos_sb[0:RH, :])
            nc.vector.tensor_mul(tb, stg[RH:2 * RH, :], sin_sb[RH:2 * RH, :])
            nc.vector.tensor_sub(dst[0:RH, :], ta, tb)
            # out[RH:2RH] = x1*cos1 + x0*sin0
            nc.vector.tensor_mul(ta, stg[RH:2 * RH, :], cos_sb[RH:2 * RH, :])
            nc.vector.tensor_mul(tb, stg[0:RH, :], sin_sb[0:RH, :])
            nc.vector.tensor_add(dst[RH:2 * RH, :], ta, tb)
            nc.vector.tensor_copy(out=dst[2 * RH:, :], in_=stg[2 * RH:, :])

        # ---------------- phase 1: norm1 ----------------
        with (
            tc.tile_pool(name="n1tmp", bufs=2) as n1tmp,
            tc.tile_pool(name="n1ps", bufs=1, space="PSUM") as n1ps,
        ):
            rmsnorm(xt_sb, h1_sb, n1tmp, n1ps)

        # tiny 8-core collective issued first: absorbs cross-core launch
        # skew while PE is still busy with norm1, so the real k/v gathers
        # see aligned peers
        sync_in = dramp.tile([1, 16], BF16, name="sync_in")
        sync_out = dramp.tile([8, 16], BF16, name="sync_out")
        nc.gpsimd.collective_compute(
            "AllGather", mybir.AluOpType.bypass,
            replica_groups=[[0, 1, 2, 3, 4, 5, 6, 7]],
            ins=[sync_in[:, :].flatten()], outs=[sync_out[:, :].flatten()],
        )

        # ------- phase 2: k/v projections, quarter all-gathers (2 kv each) -------
        k_ag = [dramp.tile([4, 2, NTT, 128, 128], BF16, name=f"kag{i}")
                for i in range(2)]
        v_ag = [dramp.tile([4, 2, NTT, 128, 128], BF16, name=f"vag{i}")
                for i in range(2)]
        k_bounce = dramp.tile([NKV, NTT, 128, 128], BF16)
        v_bounce = dramp.tile([NKV, NTT, 128, 128], BF16)

        with (
            tc.tile_pool(name="wqkv", bufs=3) as wqkvp,
            tc.tile_pool(name="qkvtmp", bufs=3) as qkvtmp,
            tc.tile_pool(name="qkvps", bufs=2, space="PSUM") as qkvps,
            tc.tile_pool(name="vps", bufs=2, space="PSUM") as vps,
        ):
            # k: per kv head -> kT [hd, t] roped -> bounce
            for kv in range(NKV):
                wk_sb = wqkvp.tile([128, NET, HD], BF16, tag="wk")
                nc.sync.dma_start(out=wk_sb, in_=wk_d[kv, :, :, :])
                kt_ps = qkvps.tile([128, TPC], F32, tag="proj")
                for e in range(NET):
                    nc.tensor.matmul(kt_ps, wk_sb[:, e, :], h1_sb[e],
                                     start=(e == 0), stop=(e == NET - 1))
                kt = qkvtmp.tile([128, TPC], BF16, tag="kt")
                rope(kt, kt_ps, qkvtmp)
                nc.gpsimd.dma_start(
                    out=k_bounce[kv, :, :, :].rearrange("a p t -> p a t"),
                    in_=kt[:, :].rearrange("p (a t) -> p a t", a=NTT))

            # v: natural layout [t, vd] tiles, direct matmul
            for kv in range(NKV):
                wv_sb = wqkvp.tile([128, NET, HD], BF16, tag="wv")
                nc.sync.dma_start(out=wv_sb, in_=wv_d[kv, :, :, :])
                for tt in range(NTT):
                    v_ps = vps.tile([128, 128], F32, tag="vps")
                    for e in range(NET):
                        nc.tensor.matmul(
                            v_ps, h1_sb[e][:, tt * 128:(tt + 1) * 128],
                            wv_sb[:, e, :],
                            start=(e == 0), stop=(e == NET - 1))
                    v_nat = qkvtmp.tile([128, 128], BF16, tag="vnat")
                    nc.scalar.copy(out=v_nat, in_=v_ps)
                    nc.scalar.dma_start(out=v_bounce[kv, tt, :, :], in_=v_nat)

            # four quarter collectives; straight-line order, early triggers
            for i in range(2):
                nc.gpsimd.collective_compute(
                    "AllGather", mybir.AluOpType.bypass, replica_groups=GROUPS,
                    ins=[k_bounce[2 * i:2 * i + 2, :, :, :].flatten()],
                    outs=[k_ag[i][:, :, :, :, :].flatten()],
                )
            for i in range(2):
                nc.gpsimd.collective_compute(
                    "AllGather", mybir.AluOpType.bypass, replica_groups=GROUPS,
                    ins=[v_bounce[2 * i:2 * i + 2, :, :, :].flatten()],
                    outs=[v_ag[i][:, :, :, :, :].flatten()],
                )

        xt_es.close()

        # ---------------- phase 3+4: q-proj fused with attention ----------------
        at_es = ExitStack()
        atp = at_es.enter_context(tc.tile_pool(name="at", bufs=1, side="left"))
        attn_sb = [atp.tile([128, TPC], BF16, name=f"at{h}") for h in range(NH)]

        with (
            tc.tile_pool(name="kvfull", bufs=1) as kvfp,
            tc.tile_pool(name="maskp", bufs=1) as maskp,
            tc.tile_pool(name="ptile", bufs=5) as ptp,
            tc.tile_pool(name="atmp", bufs=1) as atmp,
            tc.tile_pool(name="wq2", bufs=3) as wq2p,
            tc.tile_pool(name="qtmp", bufs=2) as qtmp,
            tc.tile_pool(name="qpool", bufs=4) as qpool,
            tc.tile_pool(name="lps", bufs=1, space="PSUM") as lps,
            tc.tile_pool(name="qkps", bufs=2, space="PSUM") as qkps,
            tc.tile_pool(name="avps", bufs=1, space="PSUM") as avps,
            tc.tile_pool(name="qprps", bufs=2, space="PSUM") as qprps,
        ):
            mask_all = maskp.tile([128, NST, TPC], BF16)
            nc.sync.dma_start(out=mask_all, in_=mask_d[:, :, :])
            mask_sb = [mask_all[:, s, :] for s in range(NST)]

            # gathered K/V loads on GpSimd (its only queue traffic now), so
            # nothing is head-of-line blocked behind the collectives
            kt_full = [kvfp.tile([128, CS], BF16, name=f"ktf{kv}") for kv in range(NKV)]
            v_full = [kvfp.tile([128, NST, 128], BF16, name=f"vf{kv}")
                      for kv in range(NKV)]
            for kv in range(NKV):
                for r in range(4):
                    nc.gpsimd.dma_start(
                        out=kt_full[kv][:, r * TPC:(r + 1) * TPC]
                            .rearrange("p (a t) -> p a t", a=NTT),
                        in_=k_ag[kv // 2][r, kv % 2, :, :, :]
                            .rearrange("a p t -> p a t"))
            for kv in range(NKV):
                for r in range(4):
                    nc.gpsimd.dma_start(
                        out=v_full[kv][:, r * NTT:(r + 1) * NTT, :],
                        in_=v_ag[kv // 2][r, kv % 2, :, :, :]
                            .rearrange("a t v -> t a v"))

            PIPE = 4  # QK runs this many stiles ahead of exp/PV

            # four group tiles; head h lives at partition 32*(h%4) of
            # group h//4 (DVE outputs must be 32-partition aligned)
            l_grp = [atmp.tile([128, TPC], F32, name=f"l_grp{i}")
                     for i in range(4)]

            for h in range(NH):
                kv = h // N_REP
                # q-proj for this head (fills the all-gather wait on PE)
                wq_sb = wq2p.tile([128, NET, HD], BF16, tag="wq")
                nc.sync.dma_start(out=wq_sb, in_=wq_d[h, :, :, :])
                qt_ps = qprps.tile([128, TPC], F32, tag="qproj")
                for e in range(NET):
                    nc.tensor.matmul(qt_ps, wq_sb[:, e, :], h1_sb[e],
                                     start=(e == 0), stop=(e == NET - 1))
                qt_h = qpool.tile([128, TPC], BF16, tag="qt")
                rope(qt_h, qt_ps, qtmp)
                av_ps = avps.tile([128, TPC], F32, tag="av")
                l_ps = lps.tile([1, TPC], F32, tag="l")
                qk_q, pt_q = [], []

                def issue_qk(s):
                    # stile pairs share a 2-bank psum tile so exp runs once
                    # per pair
                    if s % 2 == 0:
                        qk_q.append(qkps.tile([128, 2 * TPC], F32,
                                              tag="qk", name="qkpair"))
                    pair_ps = qk_q[-1]
                    nc.tensor.matmul(
                        pair_ps[:, (s % 2) * TPC:(s % 2 + 1) * TPC],
                        kt_full[kv][:, s * 128:(s + 1) * 128],
                        qt_h, start=True, stop=True)

                partials = {}  # binary-counter tree accumulator (DVE, bf16)

                def tree_push(t):
                    lv = 0
                    while lv in partials:
                        nx = ptp.tile([128, 2 * TPC], BF16, tag="tsum", bufs=6)
                        nc.vector.tensor_add(nx, partials.pop(lv), t)
                        t = nx
                        lv += 1
                    partials[lv] = t

                def issue_exp_pair(p):
                    pair_ps = qk_q.pop(0)
                    pair = ptp.tile([128, 2 * TPC], BF16,
                                    tag="pt", bufs=9, name="ptpair")
                    nc.scalar.activation(out=pair, in_=pair_ps, func=AF.Exp)
                    mk = mask_all[:, 2 * p:2 * p + 2, :].rearrange(
                        "p a t -> p (a t)")
                    nc.vector.tensor_mul(pair, pair, mk)
                    tree_push(pair)
                    pt_q.append(pair)

                def issue_pv(s):
                    pair = pt_q[0]
                    nc.tensor.matmul(av_ps,
                                     v_full[kv][:, s, :],
                                     pair[:, (s % 2) * TPC:(s % 2 + 1) * TPC],
                                     start=(s == 0), stop=(s == NST - 1))
                    if s % 2 == 1:
                        pt_q.pop(0)

                NPAIR = NST // 2
                PP = PIPE // 2  # pairs of lookahead
                for p in range(PP):
                    issue_qk(2 * p)
                    issue_qk(2 * p + 1)
                    issue_exp_pair(p)
                for p in range(NPAIR):
                    if p + PP < NPAIR:
                        issue_qk(2 * (p + PP))
                        issue_qk(2 * (p + PP) + 1)
                    issue_pv(2 * p)
                    issue_pv(2 * p + 1)
                    if p + PP < NPAIR:
                        issue_exp_pair(p + PP)

                # l: combine tree partials, two ones-matmuls per head
                lvls = sorted(partials)
                acc = partials[lvls[0]]
                for lv in lvls[1:]:
                    nx = ptp.tile([128, 2 * TPC], BF16, tag="tsum", bufs=6)
                    nc.vector.tensor_add(nx, partials[lv], acc)
                    acc = nx
                partials.clear()
                nc.tensor.matmul(l_ps, ones_col, acc[:, 0:TPC],
                                 start=True, stop=False)
                nc.tensor.matmul(l_ps, ones_col, acc[:, TPC:],
                                 start=False, stop=True)

                # stash raw av (bf16) and l + exp(sink); the normalize runs
                # batched after the head loop (one Ln/Exp table load total)
                nc.scalar.copy(out=attn_sb[h], in_=av_ps)
                row = 32 * (h % 4)
                nc.vector.tensor_scalar_add(
                    l_grp[h // 4][row:row + 1, :], l_ps, sink_sb[:, h:h + 1])

            # batched: rec = exp(-ln(l)), broadcast via DRAM, normalize
            rec_dram = dramp.tile([NH, TPC], F32, name="rec_dram")
            for i in range(4):
                lnl = atmp.tile([128, TPC], F32, tag="lnl", bufs=2)
                nc.scalar.activation(out=lnl, in_=l_grp[i], func=AF.Ln)
                rec_g = atmp.tile([128, TPC], F32, tag="rec_g", bufs=2)
                nc.scalar.activation(out=rec_g, in_=lnl, func=AF.Exp,
                                     scale=-1.0)
                for j in range(4):
                    nc.sync.dma_start(out=rec_dram[4 * i + j:4 * i + j + 1, :],
                                      in_=rec_g[32 * j:32 * j + 1, :])
            for h in range(NH):
                rec_bc = atmp.tile([128, TPC], F32, tag="recbcs", bufs=3)
                nc.sync.dma_start(
                    out=rec_bc,
                    in_=rec_dram[h:h + 1, :].to_broadcast([128, TPC]))
                nc.vector.tensor_mul(attn_sb[h], attn_sb[h], rec_bc)

        h1_es.close()

        # ---------------- phase 5: o proj + residual ----------------
        # xt is re-loaded from DRAM (cheaper than keeping it resident), and
        # o-proj runs as two 8-head psum groups so group A can start while
        # the second half of wo is still streaming in.
        x2_es = ExitStack()
        x2p = x2_es.enter_context(tc.tile_pool(name="x2", bufs=1, side="right"))
        x2_sb = [x2p.tile([128, TPC], F32, name=f"x2_{e}") for e in range(NET)]

        with (
            tc.tile_pool(name="xt2", bufs=1, side="left") as xt2p,
            tc.tile_pool(name="wo2", bufs=1, side="left") as wop2,
            tc.tile_pool(name="ops", bufs=2, space="PSUM") as ops,
        ):
            xt2_all = xt2p.tile([128, NET, TPC], F32)
            for g in range(4):
                nc.scalar.dma_start(
                    out=xt2_all[:, g * 4:(g + 1) * 4, :],
                    in_=xt_d[g * 512:(g + 1) * 512, :]
                        .rearrange("(et p) t -> p et t", p=128))
            wo_all = [wop2.tile([128, NET, 128], BF16, name=f"wo2_{h}")
                      for h in range(NH)]
            for h in range(NH):
                nc.sync.dma_start(out=wo_all[h], in_=wo_d[h, :, :, :])
            for e in range(NET):
                oa_ps = ops.tile([128, TPC], F32, tag="oa")
                for h in range(8):
                    nc.tensor.matmul(oa_ps, wo_all[h][:, e, :], attn_sb[h],
                                     start=(h == 0), stop=(h == 7))
                ob_ps = ops.tile([128, TPC], F32, tag="ob")
                for h in range(8, NH):
                    nc.tensor.matmul(ob_ps, wo_all[h][:, e, :], attn_sb[h],
                                     start=(h == 8), stop=(h == NH - 1))
                xa = x2p.tile([128, TPC], F32, tag="xa", bufs=2)
                nc.vector.tensor_add(xa, xt2_all[:, e, :], oa_ps)
                nc.vector.tensor_add(x2_sb[e], xa, ob_ps)

        at_es.close()

        # ---------------- phase 6: norm2 ----------------
        h2_es = ExitStack()
        h2p = h2_es.enter_context(tc.tile_pool(name="h2", bufs=1, side="left"))
        h2_sb = [h2p.tile([128, TPC], BF16, name=f"h2_{e}") for e in range(NET)]

        with (
            tc.tile_pool(name="n2tmp", bufs=2) as n2tmp,
            tc.tile_pool(name="n2ps", bufs=1, space="PSUM") as n2ps,
        ):
            rmsnorm(x2_sb, h2_sb, n2tmp, n2ps)

        # ---------------- phase 7: MLP gate/up ----------------
        a_es = ExitStack()
        amlpp = a_es.enter_context(tc.tile_pool(name="amlp", bufs=1, side="right"))
        a_sb = [amlpp.tile([128, TPC], BF16, name=f"a{f}") for f in range(NFT)]
        wdn_es = ExitStack()  # open before phase 7 so down weights prefetch
        wdnp = wdn_es.enter_context(tc.tile_pool(name="wdn", bufs=2, side="right"))
        with (
            tc.tile_pool(name="wgu", bufs=3) as wgup,
            tc.tile_pool(name="gups", bufs=2, space="PSUM") as gups,
            tc.tile_pool(name="uups", bufs=2, space="PSUM") as uups,
            tc.tile_pool(name="gtmp", bufs=3) as gtmp,
        ):
            for f in range(NFT):
                wg_sb = wgup.tile([128, NET, 128], BF16, tag="wg")
                nc.sync.dma_start(out=wg_sb, in_=gate_d[f, :, :, :])
                wu_sb = wgup.tile([128, NET, 128], BF16, tag="wu")
                nc.sync.dma_start(out=wu_sb, in_=up_d[f, :, :, :])
                g_ps = gups.tile([128, TPC], F32, tag="g")
                u_ps = uups.tile([128, TPC], F32, tag="u")
                for e in range(NET):
                    nc.tensor.matmul(g_ps, wg_sb[:, e, :], h2_sb[e],
                                     start=(e == 0), stop=(e == NET - 1))
                for e in range(NET):
                    nc.tensor.matmul(u_ps, wu_sb[:, e, :], h2_sb[e],
                                     start=(e == 0), stop=(e == NET - 1))
                sg = gtmp.tile([128, TPC], BF16, tag="sg")
                nc.scalar.activation(out=sg, in_=g_ps, func=AF.Silu)
                nc.vector.tensor_mul(a_sb[f], u_ps, sg)

        h2_es.close()

        # ---------------- phase 8: MLP down + residual + out ----------------
        with (
            tc.tile_pool(name="dps", bufs=2, space="PSUM") as dps,
            tc.tile_pool(name="otmp", bufs=3) as otmp,
        ):
            for e in range(NET):
                wd_sb = wdnp.tile([128, NFT, 128], BF16, tag="wd")
                nc.sync.dma_start(out=wd_sb, in_=down_d[e, :, :, :])
                d_ps = dps.tile([128, TPC], F32, tag="d")
                for f in range(NFT):
                    nc.tensor.matmul(d_ps, wd_sb[:, f, :], a_sb[f],
                                     start=(f == 0), stop=(f == NFT - 1))
                o_t = otmp.tile([128, TPC], F32, tag="ot")
                nc.vector.tensor_add(o_t, x2_sb[e], d_ps)
                nc.sync.dma_start(out=out_d[e * 128:(e + 1) * 128, :], in_=o_t)

        wdn_es.close()
        a_es.close()
        x2_es.close()

    nc.finalize()
    return nc


def _prep_inputs(x, segment_ids, wq, wk, wv, wo, sink_bias, gate_w, up_w, down_w,
                 norm1_scale, norm2_scale, k_cache, v_cache):
    """Host-side fold/pack/shard. Returns in_maps for 8 cores."""
    bf16 = ml_dtypes.bfloat16
    f32 = np.float32

    x = np.asarray(x, f32)
    seg = np.asarray(segment_ids)
    wq = np.asarray(wq, f32); wk = np.asarray(wk, f32); wv = np.asarray(wv, f32)
    wo = np.asarray(wo, f32)
    gate_w = np.asarray(gate_w, f32); up_w = np.asarray(up_w, f32)
    down_w = np.asarray(down_w, f32)
    s1 = (1.0 + np.asarray(norm1_scale, f32))
    s2 = (1.0 + np.asarray(norm2_scale, f32))

    wq_eff = wq * s1[:, None, None] * SCALING
    wk_eff = wk * s1[:, None, None]
    wv_eff = wv * s1[:, None, None]
    gate_eff = gate_w * s2[:, None]
    up_eff = up_w * s2[:, None]

    wq_pack = np.ascontiguousarray(
        wq_eff.reshape(NET, 128, NH, HD).transpose(2, 1, 0, 3)).astype(bf16)
    wk_pack = np.ascontiguousarray(
        wk_eff.reshape(NET, 128, NKV, HD).transpose(2, 1, 0, 3)).astype(bf16)
    wv_pack = np.ascontiguousarray(
        wv_eff.reshape(NET, 128, NKV, HD).transpose(2, 1, 0, 3)).astype(bf16)
    wo_pack = np.ascontiguousarray(wo.reshape(NH, 128, NET, 128)).astype(bf16)
    gate_pack = np.ascontiguousarray(
        gate_eff.reshape(NET, 128, NFT, 128).transpose(2, 1, 0, 3)).astype(bf16)
    up_pack = np.ascontiguousarray(
        up_eff.reshape(NET, 128, NFT, 128).transpose(2, 1, 0, 3)).astype(bf16)
    down_pack = np.ascontiguousarray(
        down_w.reshape(NFT, 128, NET, 128).transpose(2, 1, 0, 3)).astype(bf16)
    sink_exp = np.exp(np.asarray(sink_bias, f32)).reshape(1, NH).astype(f32)

    # positions / rope tables (mirrors reference semantics, cur_ind = 0)
    ar = np.arange(T, dtype=np.int64)
    nonzero = seg != 0
    first = np.argmax(nonzero, axis=1).astype(np.int64)
    positions = np.where(nonzero, ar[None, :] - first[:, None], 2 ** 30)
    fraction = np.arange(0, ROPE_DIM, 2, dtype=f32) / ROPE_DIM
    inv_freq = (1.0 / (ROPE_THETA ** fraction)).astype(f32)
    ang = positions.astype(f32)[:, :, None] * inv_freq[None, None, :]
    sin_t = np.sin(ang).astype(f32)  # [B, T, RH]
    cos_t = np.cos(ang).astype(f32)

    # attention mask (mirrors reference, cur_ind = 0)
    left_pads = np.sum(np.cumsum(nonzero, axis=-1) == 0, axis=-1).astype(np.int64)
    q_pos = ar[None, :] - left_pads[:, None]                      # [B, T]
    ts_ = np.arange(CS, dtype=np.int64)
    kv_seg = (ts_[None, :] >= left_pads[:, None]) & (ts_[None, :] < T)
    k_pos = ts_[None, :] - left_pads[:, None]                     # [B, S]
    causal = k_pos[:, None, :] <= q_pos[:, :, None]
    seg_m = kv_seg[:, None, :].astype(np.int64) == seg[:, :, None].astype(np.int64)
    window = k_pos[:, None, :] >= q_pos[:, :, None] - (WIN - 1)
    final = causal & seg_m & window                                # [B, T, S]
    mask01 = final.astype(f32)                                     # 1 keep, 0 drop

    in_maps = []
    for c in range(N_CORES):
        b, r = c // 4, c % 4
        t0 = r * TPC
        xt = np.ascontiguousarray(x[b, t0:t0 + TPC, :].T)          # [EMB, TPC]
        sin_f = np.concatenate([sin_t[b, t0:t0 + TPC, :]] * 2, axis=1).T
        cos_f = np.concatenate([cos_t[b, t0:t0 + TPC, :]] * 2, axis=1).T
        sincos = np.stack([
            np.ascontiguousarray(sin_f), np.ascontiguousarray(cos_f),
        ]).astype(bf16)                                            # [2, 64, TPC]
        # [s_in_tile, stile, t] packing of the transposed mask
        mt = mask01[b, t0:t0 + TPC, :].T.reshape(NST, 128, TPC)
        maskt = np.ascontiguousarray(mt.transpose(1, 0, 2)).astype(bf16)
        in_maps.append({
            "xt": xt, "wq": wq_pack, "wk": wk_pack, "wv": wv_pack,
            "wo": wo_pack, "gate": gate_pack, "up": up_pack, "down": down_pack,
            "sincos": sincos, "maskt": maskt, "sinkexp": sink_exp,
        })
    return in_maps


def kernel(**inputs) -> np.ndarray:
    global _BUILT, LAST_RESULT
    if _BUILT is None:
        _BUILT = _build_nc()
    nc = _BUILT

    in_maps = _prep_inputs(**inputs)

    trace = os.environ.get("BASS_KERNEL_TRACE") == "1"
    kwargs = {}
    if trace:
        kwargs["trace"] = True
        kwargs["trace_cores"] = list(range(N_CORES))
    res = bass_utils.run_bass_kernel_spmd(
        nc, in_maps, core_ids=list(range(N_CORES)), **kwargs)
    LAST_RESULT = res

    out = np.empty((B, T, EMB), np.float32)
    for c in range(N_CORES):
        b, r = c // 4, c % 4
        t0 = r * TPC
        out[b, t0:t0 + TPC, :] = res.results[c]["out"].T
    return out
